# revision 11
# baseline (speedup 1.0000x reference)
"""CombinedSparsity (spatial max-pool + lifetime top-k + max-unpool) on 8 TRN2 cores.

Strategy: shard the 128 channels across 8 cores (16 each). Per (b, c) map the
output is all zeros except (possibly) one element: the map's max, written back
at its argmax position, kept only if that max is among the top-6 over the batch
for its channel. The kernel is HBM-read-bound (33.5MB/core), so the stream is
kept saturated and everything else is hidden under or packed tightly after it:

  1. stream the shard in 2-channel groups; each group gets a TWO-LEVEL max
     reduce on DVE: HW=4096 -> 64 chunk-partials -> 1 pooled value. Partials
     are spilled to per-prep DRAM scratch (on the Act HWDGE queue).
  2. per prep-slice of channels (6/6/2/1/1), find the per-channel top-8 batch
     entries (transpose + InstMax/InstMaxIndex), compact the 6 survivors per
     channel into a shared [96, .] tile block, and indirect-gather each
     survivor's 64 partials (256B). All of this has no slow-DMA dependency,
     so it overlaps the stream; the last channel's DMA is sub-split 4x so
     only ~1us of reduce remains when the final byte lands.
  3. one merged post-stream pass over all 96 survivors: 64-wide InstMaxIndex
     -> chunk index, one 96-row indirect gather of 64-elem chunks from x,
     64-wide InstMaxIndex -> position, then a single 96-element scatter.
     Output stays zero elsewhere (PJRT output buffers are donated zero-filled).
"""
import numpy as np

import concourse.bass as bass
import concourse.bacc as bacc
import concourse.tile as tile
from concourse import mybir
from concourse.bass_utils import run_bass_kernel_spmd
from concourse.masks import make_identity

B = 128
C_FULL = 128
H = 64
W = 64
HW = H * W
N_CORES = 8
CSH = C_FULL // N_CORES      # channels per core
K = 6                        # lifetime top-k
S = 64                       # chunks per map (two-level reduce)
T = HW // S                  # elems per chunk
F32 = mybir.dt.float32
I32 = mybir.dt.int32
U32 = mybir.dt.uint32
NSURV = CSH * K              # survivors per core

# prep slices: (c_lo, c_hi). Tiny trailing preps so almost all of the top-k
# prep work overlaps the stream.
PREPS = [(0, 6), (6, 12), (12, 14), (14, 15), (15, 16)]
N_SUB = 4                    # sub-DMAs for the final channel

_nc_cache = None


def _build():
    global _nc_cache
    if _nc_cache is not None:
        return _nc_cache

    nc = bacc.Bacc("TRN2", target_bir_lowering=False, debug=False)
    x = nc.dram_tensor("x", [B, CSH, HW], F32, kind="ExternalInput")
    y = nc.dram_tensor("y", [B, CSH, HW], F32, kind="ExternalOutput")
    x64 = x.rearrange("b c (s t) -> (b c s) t", t=T)   # chunk-row view
    y_elem = y.rearrange("b c h -> (b c h)")[:, None]
    n_elem = B * CSH * HW

    with tile.TileContext(nc) as tc:
        with (
            tc.tile_pool(name="const", bufs=1) as cp,
            tc.tile_pool(name="gxp", bufs=5) as gxp,
            tc.tile_pool(name="small", bufs=1) as sp,
            tc.tile_pool(name="dram", bufs=1, space="DRAM") as dp,
            tc.tile_pool(name="ps", bufs=1, space="PSUM") as pp,
        ):
            ident0 = cp.tile([B, B], F32)
            make_identity(nc, ident0[:])
            # keep matmul inputs single-producer-engine (DVE)
            ident = cp.tile([B, B], F32)
            nc.vector.tensor_copy(out=ident[:], in_=ident0[:])

            # per-prep channel-index columns (scalar operands must live in
            # the same partitions as the op's lanes, i.e. start at 0)
            rel_cols, abs_cols = [], []
            for p, (c_lo, c_hi) in enumerate(PREPS):
                n = c_hi - c_lo
                rel_i = cp.tile([n, 1], I32, name=f"reli{p}")
                nc.gpsimd.iota(
                    rel_i[:], pattern=[[1, 1]], base=0, channel_multiplier=1
                )
                rel_f = cp.tile([n, 1], F32, name=f"relf{p}")
                nc.vector.tensor_copy(out=rel_f[:], in_=rel_i[:])
                abs_i = cp.tile([n, 1], I32, name=f"absi{p}")
                nc.gpsimd.iota(
                    abs_i[:], pattern=[[1, 1]], base=c_lo, channel_multiplier=1
                )
                abs_f = cp.tile([n, 1], F32, name=f"absf{p}")
                nc.vector.tensor_copy(out=abs_f[:], in_=abs_i[:])
                rel_cols.append(rel_f)
                abs_cols.append(abs_f)

            # shared survivor-indexed tiles, one partition-slice per prep
            # (only DMAs touch the partition-offset slices; DVE ops must
            # start at partition 0)
            cpk_all = sp.tile([NSURV, 3], F32)    # (scratch_row, x_row, value)
            cp64_all = sp.tile([NSURV, S], F32)   # survivor's 64 partials

            def emit_prep(p, c_lo, c_hi):
                n = c_hi - c_lo
                partials = sp.tile([B, n * S], F32, name=f"part{p}")
                pooled = sp.tile([B, n], F32, name=f"pooled{p}")
                scratch = dp.tile([B, n, S], F32, name=f"scr{p}")

                def finish_group(crel, ncols):
                    nc.vector.tensor_reduce(
                        out=pooled[:, crel:crel + ncols],
                        in_=partials[:, crel * S:(crel + ncols) * S].rearrange(
                            "p (c s) -> p c s", c=ncols
                        ),
                        axis=mybir.AxisListType.X,
                        op=mybir.AluOpType.max,
                    )
                    nc.scalar.dma_start(
                        out=scratch[:, crel:crel + ncols, :],
                        in_=partials[:, crel * S:(crel + ncols) * S],
                    )

                def load_group(crel, ncols):
                    c0 = c_lo + crel
                    gx = gxp.tile([B, ncols * HW], F32, tag="gx")
                    nc.sync.dma_start(out=gx[:], in_=x[:, c0:c0 + ncols, :])
                    nc.vector.tensor_reduce(
                        out=partials[:, crel * S:(crel + ncols) * S],
                        in_=gx[:].rearrange(
                            "p (c s t) -> p c s t", c=ncols, s=S
                        ),
                        axis=mybir.AxisListType.X,
                        op=mybir.AluOpType.max,
                    )
                    finish_group(crel, ncols)

                # ---- streaming ----
                if p < len(PREPS) - 1:
                    for crel in range(0, n, 2):
                        load_group(crel, min(2, n - crel))
                else:
                    # final channel: sub-split so its reduce isn't exposed
                    sub = HW // N_SUB
                    for j in range(N_SUB):
                        gx = gxp.tile([B, sub], F32, tag="gx")
                        nc.sync.dma_start(
                            out=gx[:], in_=x[:, c_lo:c_lo + 1,
                                             j * sub:(j + 1) * sub]
                        )
                        nc.vector.tensor_reduce(
                            out=partials[:, j * (sub // T):(j + 1) * (sub // T)],
                            in_=gx[:].rearrange("p (s t) -> p s t", t=T),
                            axis=mybir.AxisListType.X,
                            op=mybir.AluOpType.max,
                        )
                    finish_group(0, 1)

                # ---- prep: top-8 over batch, compact, gather partials ----
                lo, hi = c_lo * K, c_hi * K
                pooled_t_ps = pp.tile([n, B], F32, name=f"ptps{p}")
                nc.tensor.transpose(
                    out=pooled_t_ps[:], in_=pooled[:], identity=ident[:]
                )
                pooled_t = sp.tile([n, B], F32, name=f"pt{p}")
                nc.scalar.copy(out=pooled_t[:], in_=pooled_t_ps[:])

                pt8 = sp.tile([n, 8], F32, name=f"pt8{p}")
                nc.vector.max(out=pt8[:], in_=pooled_t[:])
                pi8 = sp.tile([n, 8], U32, name=f"pi8{p}")
                nc.vector.max_index(
                    out=pi8[:], in_max=pt8[:], in_values=pooled_t[:]
                )
                pi8f = sp.tile([n, 8], F32, name=f"pi8f{p}")
                nc.vector.tensor_copy(out=pi8f[:], in_=pi8[:])

                pk = sp.tile([n, 8 * 3], F32, name=f"pk{p}")
                pkv = pk[:].rearrange("q (j k) -> q j k", k=3)
                nc.vector.tensor_scalar(
                    out=pkv[:, :, 0:1], in0=pi8f[:], scalar1=float(n),
                    scalar2=rel_cols[p][:, 0:1],
                    op0=mybir.AluOpType.mult, op1=mybir.AluOpType.add,
                )
                nc.vector.tensor_scalar(
                    out=pkv[:, :, 1:2], in0=pi8f[:], scalar1=float(CSH),
                    scalar2=abs_cols[p][:, 0:1],
                    op0=mybir.AluOpType.mult, op1=mybir.AluOpType.add,
                )
                nc.scalar.copy(out=pkv[:, :, 2:3], in_=pt8[:])

                nc.gpsimd.dma_start(
                    out=cpk_all[lo:hi, :], in_=pkv[:, 0:K, :]
                )
                cru_f = sp.tile([hi - lo, 1], F32, name=f"cruf{p}")
                nc.scalar.dma_start(out=cru_f[:], in_=pkv[:, 0:K, 0:1])
                cru_i = sp.tile([hi - lo, 1], I32, name=f"crui{p}")
                nc.vector.tensor_copy(out=cru_i[:], in_=cru_f[:])
                nc.gpsimd.indirect_dma_start(
                    out=cp64_all[lo:hi, :], out_offset=None,
                    in_=scratch[:].rearrange("b c s -> (b c) s"),
                    in_offset=bass.IndirectOffsetOnAxis(
                        ap=cru_i[:, 0:1], axis=0
                    ),
                )

            for p, (c_lo, c_hi) in enumerate(PREPS):
                emit_prep(p, c_lo, c_hi)

            # ---- merged tail over all 96 survivors ----
            vb_all = sp.tile([NSURV, 8], F32)     # needle: value, 8-wide
            nc.vector.tensor_copy(
                out=vb_all[:], in_=cpk_all[:, 2:3].to_broadcast([NSURV, 8])
            )
            jc8 = sp.tile([NSURV, 8], U32)
            nc.vector.max_index(out=jc8[:], in_max=vb_all[:], in_values=cp64_all[:])
            jcf = sp.tile([NSURV, 1], F32)
            nc.vector.tensor_copy(out=jcf[:], in_=jc8[:, 0:1])

            rows2 = sp.tile([NSURV, 1], F32)
            nc.vector.tensor_scalar(
                out=rows2[:], in0=cpk_all[:, 1:2], scalar1=float(S),
                scalar2=jcf[:, 0:1],
                op0=mybir.AluOpType.mult, op1=mybir.AluOpType.add,
            )
            rows2_i = sp.tile([NSURV, 1], I32)
            nc.vector.tensor_copy(out=rows2_i[:], in_=rows2[:])
            ck = sp.tile([NSURV, T], F32)
            nc.gpsimd.indirect_dma_start(
                out=ck[:], out_offset=None,
                in_=x64[:],
                in_offset=bass.IndirectOffsetOnAxis(ap=rows2_i[:, 0:1], axis=0),
            )
            t8 = sp.tile([NSURV, 8], U32)
            nc.vector.max_index(out=t8[:], in_max=vb_all[:], in_values=ck[:])
            tf = sp.tile([NSURV, 1], F32)
            nc.vector.tensor_copy(out=tf[:], in_=t8[:, 0:1])

            off_f = sp.tile([NSURV, 1], F32)
            nc.vector.tensor_scalar(
                out=off_f[:], in0=rows2[:], scalar1=float(T),
                scalar2=tf[:, 0:1],
                op0=mybir.AluOpType.mult, op1=mybir.AluOpType.add,
            )
            off_i = sp.tile([NSURV, 1], I32)
            nc.vector.tensor_copy(out=off_i[:], in_=off_f[:])

            nc.gpsimd.indirect_dma_start(
                out=y_elem[:],
                out_offset=bass.IndirectOffsetOnAxis(ap=off_i[:, 0:1], axis=0),
                in_=cpk_all[:, 2:3],
                in_offset=None,
                bounds_check=n_elem - 1,
                oob_is_err=False,
            )

    nc.finalize()
    _nc_cache = nc
    return nc


def _install_profile_hook():
    """Inject the antenv.axon_hooks shim so trace=True captures NTFFs."""
    import sys
    import types

    if "antenv.axon_hooks" in sys.modules:
        return
    import antenv
    import trn_agent_boot.trn_boot as tb

    mod = types.ModuleType("antenv.axon_hooks")
    mod._hook = tb._ntff_profile_via_ctypes("/opt/axon/libaxon_pjrt.so")
    mod.get_axon_ntff_profile_hook = lambda: mod._hook
    mod.set_axon_ntff_profile_hook = lambda h: setattr(mod, "_hook", h)
    sys.modules["antenv.axon_hooks"] = mod
    antenv.axon_hooks = mod

    # no S3 in this container — keep artifacts local
    import concourse.bass_utils as bu

    bu.upload_artifacts = lambda tmpdir: tmpdir


def run(activations, trace=False):
    if trace:
        _install_profile_hook()
    act = np.asarray(activations)
    assert act.shape == (B, C_FULL, H, W), act.shape
    act = act.astype(np.float32, copy=False)
    nc = _build()
    in_maps = [
        {"x": np.ascontiguousarray(act[:, i * CSH:(i + 1) * CSH]).reshape(B, CSH, HW)}
        for i in range(N_CORES)
    ]
    res = run_bass_kernel_spmd(
        nc, in_maps, core_ids=list(range(N_CORES)), trace=trace
    )
    out = np.concatenate(
        [r["y"].reshape(B, CSH, H, W) for r in res.results], axis=1
    )
    return out, res


def kernel(activations):
    out, _ = run(activations, trace=False)
    return out


# revision 14
# speedup vs baseline: 1.0196x; 1.0196x over previous
"""CombinedSparsity (spatial max-pool + lifetime top-k + max-unpool) on 8 TRN2 cores.

Strategy: shard the 128 channels across 8 cores (16 each). Per (b, c) map the
output is all zeros except (possibly) one element: the map's max, written back
at its argmax position, kept only if that max is among the top-6 over the batch
for its channel. The kernel is HBM-read-bound (33.5MB/core), so the stream is
kept saturated and everything else is hidden under or packed tightly after it:

  1. stream the shard in 2-channel groups; each group gets a TWO-LEVEL max
     reduce on DVE: HW=4096 -> 64 chunk-partials -> 1 pooled value. Partials
     are spilled to per-prep DRAM scratch (on the Act HWDGE queue).
  2. per prep-slice of channels (6/6/2/1/1), find the per-channel top-8 batch
     entries (transpose + InstMax/InstMaxIndex), compact the 6 survivors per
     channel into a shared [96, .] tile block, and indirect-gather each
     survivor's 64 partials (256B). All of this has no slow-DMA dependency,
     so it overlaps the stream; the last channel's DMA is sub-split 4x so
     only ~1us of reduce remains when the final byte lands.
  3. one merged post-stream pass over all 96 survivors: 64-wide InstMaxIndex
     -> chunk index, one 96-row indirect gather of 64-elem chunks from x,
     64-wide InstMaxIndex -> position, then a single 96-element scatter.
     Output stays zero elsewhere (PJRT output buffers are donated zero-filled).
"""
import numpy as np

import concourse.bass as bass
import concourse.bacc as bacc
import concourse.tile as tile
from concourse import mybir
from concourse.bass_utils import run_bass_kernel_spmd
from concourse.masks import make_identity

B = 128
C_FULL = 128
H = 64
W = 64
HW = H * W
N_CORES = 8
CSH = C_FULL // N_CORES      # channels per core
K = 6                        # lifetime top-k
S = 64                       # chunks per map (two-level reduce)
T = HW // S                  # elems per chunk
F32 = mybir.dt.float32
I32 = mybir.dt.int32
U32 = mybir.dt.uint32
NSURV = CSH * K              # survivors per core

# prep slices: (c_lo, c_hi). Tiny trailing preps so almost all of the top-k
# prep work overlaps the stream.
PREPS = [(0, 6), (6, 12), (12, 14), (14, 15), (15, 16)]
N_SUB = 4                    # sub-DMAs for the final channel

_nc_cache = None


def _build():
    global _nc_cache
    if _nc_cache is not None:
        return _nc_cache

    nc = bacc.Bacc("TRN2", target_bir_lowering=False, debug=False)
    x = nc.dram_tensor("x", [B, CSH, HW], F32, kind="ExternalInput")
    y = nc.dram_tensor("y", [B, CSH, HW], F32, kind="ExternalOutput")
    x64 = x.rearrange("b c (s t) -> (b c s) t", t=T)   # chunk-row view
    y_elem = y.rearrange("b c h -> (b c h)")[:, None]
    n_elem = B * CSH * HW

    with tile.TileContext(nc) as tc:
        with (
            tc.tile_pool(name="const", bufs=1) as cp,
            tc.tile_pool(name="gxp", bufs=5) as gxp,
            tc.tile_pool(name="small", bufs=1) as sp,
            tc.tile_pool(name="dram", bufs=1, space="DRAM") as dp,
            tc.tile_pool(name="ps", bufs=1, space="PSUM") as pp,
        ):
            ident0 = cp.tile([B, B], F32)
            make_identity(nc, ident0[:])
            # keep matmul inputs single-producer-engine (DVE)
            ident = cp.tile([B, B], F32)
            nc.vector.tensor_copy(out=ident[:], in_=ident0[:])

            # per-prep channel-index columns (scalar operands must live in
            # the same partitions as the op's lanes, i.e. start at 0)
            rel_cols, abs_cols = [], []
            for p, (c_lo, c_hi) in enumerate(PREPS):
                n = c_hi - c_lo
                rel_i = cp.tile([n, 1], I32, name=f"reli{p}")
                nc.gpsimd.iota(
                    rel_i[:], pattern=[[1, 1]], base=0, channel_multiplier=1
                )
                rel_f = cp.tile([n, 1], F32, name=f"relf{p}")
                nc.vector.tensor_copy(out=rel_f[:], in_=rel_i[:])
                abs_i = cp.tile([n, 1], I32, name=f"absi{p}")
                nc.gpsimd.iota(
                    abs_i[:], pattern=[[1, 1]], base=c_lo, channel_multiplier=1
                )
                abs_f = cp.tile([n, 1], F32, name=f"absf{p}")
                nc.vector.tensor_copy(out=abs_f[:], in_=abs_i[:])
                rel_cols.append(rel_f)
                abs_cols.append(abs_f)

            # shared survivor-indexed tiles, one partition-slice per prep
            # (only DMAs touch the partition-offset slices; DVE ops must
            # start at partition 0)
            cpk_all = sp.tile([NSURV, 3], F32)    # (scratch_row, x_row, value)
            cp64_all = sp.tile([NSURV, S], F32)   # survivor's 64 partials

            def emit_prep(p, c_lo, c_hi):
                n = c_hi - c_lo
                partials = sp.tile([B, n * S], F32, name=f"part{p}")
                pooled = sp.tile([B, n], F32, name=f"pooled{p}")
                scratch = dp.tile([B, n, S], F32, name=f"scr{p}")

                def finish_group(crel, ncols):
                    nc.vector.tensor_reduce(
                        out=pooled[:, crel:crel + ncols],
                        in_=partials[:, crel * S:(crel + ncols) * S].rearrange(
                            "p (c s) -> p c s", c=ncols
                        ),
                        axis=mybir.AxisListType.X,
                        op=mybir.AluOpType.max,
                    )

                def load_group(crel, ncols):
                    c0 = c_lo + crel
                    gx = gxp.tile([B, ncols * HW], F32, tag="gx")
                    nc.sync.dma_start(out=gx[:], in_=x[:, c0:c0 + ncols, :])
                    nc.vector.tensor_reduce(
                        out=partials[:, crel * S:(crel + ncols) * S],
                        in_=gx[:].rearrange(
                            "p (c s t) -> p c s t", c=ncols, s=S
                        ),
                        axis=mybir.AxisListType.X,
                        op=mybir.AluOpType.max,
                    )
                    finish_group(crel, ncols)

                # ---- streaming ----
                if p < len(PREPS) - 1:
                    for crel in range(0, n, 2):
                        load_group(crel, min(2, n - crel))
                else:
                    # final channel: sub-split so its reduce isn't exposed
                    sub = HW // N_SUB
                    for j in range(N_SUB):
                        gx = gxp.tile([B, sub], F32, tag="gx")
                        nc.sync.dma_start(
                            out=gx[:], in_=x[:, c_lo:c_lo + 1,
                                             j * sub:(j + 1) * sub]
                        )
                        nc.vector.tensor_reduce(
                            out=partials[:, j * (sub // T):(j + 1) * (sub // T)],
                            in_=gx[:].rearrange("p (s t) -> p s t", t=T),
                            axis=mybir.AxisListType.X,
                            op=mybir.AluOpType.max,
                        )
                    finish_group(0, 1)

                # spill all of this prep's partials in one contiguous write
                nc.scalar.dma_start(out=scratch[:], in_=partials[:])

                # ---- prep: top-8 over batch, compact, gather partials ----
                lo, hi = c_lo * K, c_hi * K
                pooled_t_ps = pp.tile([n, B], F32, name=f"ptps{p}")
                nc.tensor.transpose(
                    out=pooled_t_ps[:], in_=pooled[:], identity=ident[:]
                )
                pooled_t = sp.tile([n, B], F32, name=f"pt{p}")
                nc.scalar.copy(out=pooled_t[:], in_=pooled_t_ps[:])

                pt8 = sp.tile([n, 8], F32, name=f"pt8{p}")
                nc.vector.max(out=pt8[:], in_=pooled_t[:])
                pi8 = sp.tile([n, 8], U32, name=f"pi8{p}")
                nc.vector.max_index(
                    out=pi8[:], in_max=pt8[:], in_values=pooled_t[:]
                )
                pi8f = sp.tile([n, 8], F32, name=f"pi8f{p}")
                nc.vector.tensor_copy(out=pi8f[:], in_=pi8[:])

                pk = sp.tile([n, 8 * 3], F32, name=f"pk{p}")
                pkv = pk[:].rearrange("q (j k) -> q j k", k=3)
                nc.vector.tensor_scalar(
                    out=pkv[:, :, 0:1], in0=pi8f[:], scalar1=float(n),
                    scalar2=rel_cols[p][:, 0:1],
                    op0=mybir.AluOpType.mult, op1=mybir.AluOpType.add,
                )
                nc.vector.tensor_scalar(
                    out=pkv[:, :, 1:2], in0=pi8f[:], scalar1=float(CSH),
                    scalar2=abs_cols[p][:, 0:1],
                    op0=mybir.AluOpType.mult, op1=mybir.AluOpType.add,
                )
                nc.scalar.copy(out=pkv[:, :, 2:3], in_=pt8[:])

                nc.gpsimd.dma_start(
                    out=cpk_all[lo:hi, :], in_=pkv[:, 0:K, :]
                )
                cru_f = sp.tile([hi - lo, 1], F32, name=f"cruf{p}")
                nc.scalar.dma_start(out=cru_f[:], in_=pkv[:, 0:K, 0:1])
                # cast on GpSimd: this waits on a small DMA that can sit
                # behind bulk stream traffic for ~8us — must not block DVE
                cru_i = sp.tile([hi - lo, 1], I32, name=f"crui{p}")
                nc.gpsimd.tensor_copy(out=cru_i[:], in_=cru_f[:])
                nc.gpsimd.indirect_dma_start(
                    out=cp64_all[lo:hi, :], out_offset=None,
                    in_=scratch[:].rearrange("b c s -> (b c) s"),
                    in_offset=bass.IndirectOffsetOnAxis(
                        ap=cru_i[:, 0:1], axis=0
                    ),
                )

            for p, (c_lo, c_hi) in enumerate(PREPS):
                emit_prep(p, c_lo, c_hi)

            # ---- merged tail over all 96 survivors ----
            vb_all = sp.tile([NSURV, 8], F32)     # needle: value, 8-wide
            nc.vector.tensor_copy(
                out=vb_all[:], in_=cpk_all[:, 2:3].to_broadcast([NSURV, 8])
            )
            jc8 = sp.tile([NSURV, 8], U32)
            nc.vector.max_index(out=jc8[:], in_max=vb_all[:], in_values=cp64_all[:])
            jcf = sp.tile([NSURV, 1], F32)
            nc.vector.tensor_copy(out=jcf[:], in_=jc8[:, 0:1])

            rows2 = sp.tile([NSURV, 1], F32)
            nc.vector.tensor_scalar(
                out=rows2[:], in0=cpk_all[:, 1:2], scalar1=float(S),
                scalar2=jcf[:, 0:1],
                op0=mybir.AluOpType.mult, op1=mybir.AluOpType.add,
            )
            rows2_i = sp.tile([NSURV, 1], I32)
            nc.vector.tensor_copy(out=rows2_i[:], in_=rows2[:])
            ck = sp.tile([NSURV, T], F32)
            nc.gpsimd.indirect_dma_start(
                out=ck[:], out_offset=None,
                in_=x64[:],
                in_offset=bass.IndirectOffsetOnAxis(ap=rows2_i[:, 0:1], axis=0),
            )
            t8 = sp.tile([NSURV, 8], U32)
            nc.vector.max_index(out=t8[:], in_max=vb_all[:], in_values=ck[:])
            tf = sp.tile([NSURV, 1], F32)
            nc.vector.tensor_copy(out=tf[:], in_=t8[:, 0:1])

            off_f = sp.tile([NSURV, 1], F32)
            nc.vector.tensor_scalar(
                out=off_f[:], in0=rows2[:], scalar1=float(T),
                scalar2=tf[:, 0:1],
                op0=mybir.AluOpType.mult, op1=mybir.AluOpType.add,
            )
            off_i = sp.tile([NSURV, 1], I32)
            nc.vector.tensor_copy(out=off_i[:], in_=off_f[:])

            nc.gpsimd.indirect_dma_start(
                out=y_elem[:],
                out_offset=bass.IndirectOffsetOnAxis(ap=off_i[:, 0:1], axis=0),
                in_=cpk_all[:, 2:3],
                in_offset=None,
                bounds_check=n_elem - 1,
                oob_is_err=False,
            )

    nc.finalize()
    _nc_cache = nc
    return nc


def _install_profile_hook():
    """Inject the antenv.axon_hooks shim so trace=True captures NTFFs."""
    import sys
    import types

    if "antenv.axon_hooks" in sys.modules:
        return
    import antenv
    import trn_agent_boot.trn_boot as tb

    mod = types.ModuleType("antenv.axon_hooks")
    mod._hook = tb._ntff_profile_via_ctypes("/opt/axon/libaxon_pjrt.so")
    mod.get_axon_ntff_profile_hook = lambda: mod._hook
    mod.set_axon_ntff_profile_hook = lambda h: setattr(mod, "_hook", h)
    sys.modules["antenv.axon_hooks"] = mod
    antenv.axon_hooks = mod

    # no S3 in this container — keep artifacts local
    import concourse.bass_utils as bu

    bu.upload_artifacts = lambda tmpdir: tmpdir


def run(activations, trace=False):
    if trace:
        _install_profile_hook()
    act = np.asarray(activations)
    assert act.shape == (B, C_FULL, H, W), act.shape
    act = act.astype(np.float32, copy=False)
    nc = _build()
    in_maps = [
        {"x": np.ascontiguousarray(act[:, i * CSH:(i + 1) * CSH]).reshape(B, CSH, HW)}
        for i in range(N_CORES)
    ]
    res = run_bass_kernel_spmd(
        nc, in_maps, core_ids=list(range(N_CORES)), trace=trace
    )
    out = np.concatenate(
        [r["y"].reshape(B, CSH, H, W) for r in res.results], axis=1
    )
    return out, res


def kernel(activations):
    out, _ = run(activations, trace=False)
    return out
